# revision 13
# baseline (speedup 1.0000x reference)
"""Trainium2 Bass kernel for nn_Attention_36361193128703 (self-contained).

Entry point: kernel(**inputs) -> np.ndarray
  inputs: x (2,2048,1024) f32, w_in (3072,1024) f32,
          kernel_offsets/amplitudes/sharpness (16,16) f32
  returns: (2, 2048, 1024) f32 attention output (matches reference).

Distribution: 8 NeuronCores = data-parallel over batch (2) x tensor-parallel
over heads (4 head-groups of 4). Each core runs an identical single-core Bass
program on its shard; outputs are concatenated on the host. No collectives.

Core pipeline (per core: 4 heads = 2 head-pairs, L=2048):
  - TISA scores (reversed) -> g = exp(bias) staged in DRAM; 128 shifted rows
    per head loaded with positive-stride DMA (Toeplitz expansion); the reversal
    moves to a stride -1 free-dim read in the bias multiply.
  - Projections in bf16 (fp8 tested: weight-quantization noise amplifies to
    ~4% output error, over the 2e-2 tolerance).
  - S = k^T q for a head-pair: two K=64 matmuls packed concurrently into PE
    row-groups 0-63 / 64-127, one [128, 2, 512] PSUM pair tile.
  - P = exp(S/8) * g: one ACT exp (FD=1024) + one DVE multiply (2x mode).
  - O = V^T P accumulated in PSUM with a ones-column denominator row.
  - Epilogue: PE transpose to [query, 64], DVE reciprocal+scale, DMA out.
"""
from contextlib import ExitStack

import numpy as np

import concourse.bass as bass
import concourse.mybir as mybir
import concourse.tile as tile
from concourse import bacc
from concourse.bass import AP
from concourse.masks import make_identity

F32 = mybir.dt.float32
BF16 = mybir.dt.bfloat16
FP8 = mybir.dt.float8e4
DR = mybir.MatmulPerfMode.DoubleRow

L = 2048
DM = 1024
HL = 4            # local heads
HD = 64
M = 4608          # padded score length (>= 2L-1 = 4095), 9 x 512 chunks
GW = 4096         # g window width per head (max needed index 4094)
IC = 512          # i-chunk (query) width per unit
NCH = L // IC     # 4 chunks
JT = 128          # j-tile (key) height
NJT = L // JT     # 16
NDC = DM // 128   # 8 d-chunks
NDP = NDC // 2    # 4 fp8 DoubleRow d-chunk pairs

# 1/sqrt(HD) is folded into the exp scale rather than the q weights.
QK_SCALE = 0.125
ONES_VAL = 1.0


def build_kernel() -> bacc.Bacc:
    nc = bacc.Bacc("TRN2", target_bir_lowering=False, debug=False, num_devices=8)

    xT_d = nc.dram_tensor("xT", [128, NDC, L], BF16, kind="ExternalInput")
    w_d = nc.dram_tensor("wkqv", [128, NDC, 768], BF16, kind="ExternalInput")
    tisa_d = nc.dram_tensor("tisa", [64, 6], F32, kind="ExternalInput")
    ramp_d = nc.dram_tensor("ramp", [64, M], BF16, kind="ExternalInput")
    out_d = nc.dram_tensor("out", [L, 256], F32, kind="ExternalOutput")

    dma_engines = [nc.sync, nc.gpsimd]

    def dma(i, out, in_):
        dma_engines[i % len(dma_engines)].dma_start(out, in_)

    with tile.TileContext(nc) as tc, ExitStack() as ctx:
        const_pool = ctx.enter_context(tc.tile_pool(name="const", bufs=1))

        aux_psum = ctx.enter_context(tc.tile_pool(name="auxps", bufs=2, space="PSUM"))
        s_psum = ctx.enter_context(tc.tile_pool(name="sps", bufs=2, space="PSUM"))
        o_psum = ctx.enter_context(tc.tile_pool(name="ops", bufs=1, space="PSUM"))

        gdram_pool = ctx.enter_context(tc.tile_pool(name="gdram", bufs=1, space="DRAM"))
        g_pad = gdram_pool.tile([HL * M], BF16)
        gbase = g_pad[:]

        # ---------------- input DMAs -------------------------------------------
        # tisa first: it is tiny and gates the whole phase-0 chain.
        tp = ctx.enter_context(tc.tile_pool(name="tisa_tmp", bufs=1))
        tisa_sb = tp.tile([64, 6], F32)
        nc.sync.dma_start(tisa_sb[:, :], tisa_d[:, :])
        rsb = tp.tile([64, M], BF16, tag="ramp")
        for rc_ in range(3):
            sl = slice(rc_ * (M // 3), (rc_ + 1) * (M // 3))
            nc.gpsimd.dma_start(rsb[:, sl], ramp_d[:, sl])
        xpool = ctx.enter_context(tc.tile_pool(name="xT", bufs=1))
        wpool = ctx.enter_context(tc.tile_pool(name="w", bufs=1))
        xT_sb = xpool.tile([128, NDC, L], BF16, name="xT", tag="xT")
        w_sb = wpool.tile([128, NDC, 768], BF16, name="w", tag="w")
        for wc in range(4):
            dma(wc, w_sb[:, 2 * wc:2 * wc + 2, :], w_d[:, 2 * wc:2 * wc + 2, :])
        for dc in range(NDC):
            dma(dc, xT_sb[:, dc:dc + 1, 0:L // 2],
                xT_d[:, dc:dc + 1, 0:L // 2])
        for dc in range(NDC):
            dma(dc + 1, xT_sb[:, dc:dc + 1, L // 2:L],
                xT_d[:, dc:dc + 1, L // 2:L])

        # ---------------- Phase 0: TISA scores (reversed) -> g_pad ---------------
        # ev[:, m] = (L-1) - m so g_pad holds rev[m] = score[2L-2 - m]; the
        # Toeplitz flip then needs only positive DMA strides.
        # tisa col0 = -off, col1 = -|sharp| (negated on the host), so the ACT
        # affine stage does the whole pointwise chain in two table-shared ops:
        # u = (ramp + (-off))^2, then evb = exp((-|sh|) * u).
        ev = tp.tile([64, M], F32, tag="scr")
        evb = tp.tile([64, M], BF16, tag="scrb")
        ampb = tp.tile([64, 4], BF16)
        nc.vector.tensor_copy(ampb[:, :], tisa_sb[:, 2:6])
        NPC = 3
        CH = M // NPC
        for cc in range(NPC):
            sl = slice(cc * CH, (cc + 1) * CH)
            nc.scalar.activation(ev[:, sl], rsb[:, sl],
                                 mybir.ActivationFunctionType.Square,
                                 bias=tisa_sb[:, 0:1])
        for cc in range(NPC):
            sl = slice(cc * CH, (cc + 1) * CH)
            nc.scalar.activation(evb[:, sl], ev[:, sl],
                                 mybir.ActivationFunctionType.Exp,
                                 scale=tisa_sb[:, 1:2])

        def emit_phase0_mms():
            for mc in range(M // 512):
                ps = aux_psum.tile([128, 512], F32, tag="aux", name="ph0")
                nc.tensor.matmul(ps[0:HL, :], ampb[:, :],
                                 evb[:, mc * 512:(mc + 1) * 512],
                                 start=True, stop=True)
                gch = tp.tile([HL, 512], BF16, tag=f"gch{mc % 2}")
                nc.scalar.activation(gch[:, :], ps[0:HL, :],
                                     mybir.ActivationFunctionType.Exp)
                dst = AP(gbase.tensor, gbase.offset + mc * 512,
                         [[M, HL], [1, 512]])
                dma(mc, dst, gch[:, :])

        # ---------------- projections ------------------------------------------
        kq_pool = ctx.enter_context(tc.tile_pool(name="kq", bufs=1))
        v_pool = ctx.enter_context(tc.tile_pool(name="V", bufs=1))
        kq_sb = [[kq_pool.tile([128, 512], BF16, name=f"kq{i}_{t}",
                          tag=f"kq{i}_{t}") for t in range(4)]
                 for i in range(4)]
        v_sb = [None] * NJT

        def emit_kq_tcn(ec, tcn):
            ps = aux_psum.tile([128, 512], F32, tag="aux", name="ps")
            for k in range(NDC):
                dc = (tcn * 2 + k) % NDC
                nc.tensor.matmul(ps[:, :],
                                 w_sb[:, dc:dc + 1, ec * 128:(ec + 1) * 128],
                                 xT_sb[:, dc:dc + 1, tcn * 512:(tcn + 1) * 512],
                                 start=(k == 0), stop=(k == NDC - 1))
            nc.vector.tensor_copy(kq_sb[ec][tcn][:, :], ps[:, :])

        def emit_vproj_tt(tt):
            ps = aux_psum.tile([128, 512], F32, tag="aux", name="ps")
            for dc in range(NDC):
                nc.tensor.matmul(ps[:, 0:256],
                                 xT_sb[:, dc:dc + 1, tt * 128:(tt + 1) * 128],
                                 w_sb[:, dc:dc + 1, 512:768],
                                 start=(dc == 0), stop=(dc == NDC - 1))
            vt = v_pool.tile([128, HL, 65], BF16, name=f"v{tt}", tag=f"v{tt}")
            psa = ps[:, 0:256]
            ps3 = AP(psa.tensor, psa.offset, [psa.ap[0], [64, HL], [1, 64]])
            nc.vector.tensor_copy(vt[:, :, 0:64], ps3)
            nc.gpsimd.memset(vt[:, :, 64:65], ONES_VAL)
            v_sb[tt] = vt

        # k/q for head-pair 0 tokens 0-511 gate the first unit; everything else
        # interleaves into the unit stream (see schedule below).
        emit_kq_tcn(0, 0)
        emit_kq_tcn(2, 0)
        emit_phase0_mms()
        for tt in range(4):
            emit_vproj_tt(tt)
        ident = const_pool.tile([128, 128], F32)
        make_identity(nc, ident[:, :])

        # ---------------- Phase 2: g windows (positive-stride DMA) ---------------
        grep_pool = ctx.enter_context(tc.tile_pool(name="grep", bufs=1))
        grepp = []
        for pp in range(2):
            gt = grep_pool.tile([128, 2, GW], BF16, name=f"grep{pp}",
                                tag=f"grep{pp}")
            grepp.append(gt)
        for hi in range(HL):
            src = AP(gbase.tensor, gbase.offset + hi * M,
                     [[1, 128], [1, GW]])
            dma(hi, grepp[hi // 2][:, hi % 2:hi % 2 + 1, :], src)

        # ---------------- Phase 3: attention units -------------------------------
        p_pool = ctx.enter_context(tc.tile_pool(name="p", bufs=6))
        e_pool = ctx.enter_context(tc.tile_pool(name="es", bufs=6))
        o_pool = ctx.enter_context(tc.tile_pool(name="o", bufs=4))
        r_pool = ctx.enter_context(tc.tile_pool(name="r", bufs=2))
        out_pool = ctx.enter_context(tc.tile_pool(name="out", bufs=2))

        def emit_S(pp, c, jt):
            """Row-packed head-pair S matmuls -> [128, 2, 512] PSUM tile."""
            ps = s_psum.tile([128, 2, IC], F32, tag="S", name=f"s{pp}_{c}_{jt}")
            kqt = kq_sb[pp][jt // 4]
            qqt = kq_sb[2 + pp][c]
            joff = (jt % 4) * JT
            for h in range(2):
                pb = h * 64
                nc.tensor.matmul(ps[:, h:h + 1, :],
                                 kqt[pb:pb + 64, joff:joff + JT],
                                 qqt[pb:pb + 64, :],
                                 start=True, stop=True)
            return ps

        def emit_expmult(pp, c, jt, ps, eng):
            es = e_pool.tile([128, 2, IC], BF16, tag="es", name=f"e{pp}_{c}_{jt}")
            nc.scalar.activation(es[:, :, :], ps[:, :, :],
                                 mybir.ActivationFunctionType.Exp,
                                 scale=QK_SCALE)
            pt = p_pool.tile([128, 2, IC], BF16, tag="p", name=f"p{pp}_{c}_{jt}")
            u0 = (L - 1) - c * IC + jt * JT
            g2 = grepp[pp][:, :, :]
            g_rev = AP(g2.tensor, g2.offset + u0, [g2.ap[0], [GW, 2], [-1, IC]])
            eng.tensor_mul(pt[:, :, :], es[:, :, :], g_rev)
            return pt

        psos = {}

        def emit_AV(pp, c, jt, pt):
            if jt == 0:
                psos[0] = o_psum.tile([65, IC], F32, tag="O0", name=f"o0_{pp}_{c}")
                psos[1] = o_psum.tile([65, IC], F32, tag="O1", name=f"o1_{pp}_{c}")
            for h in range(2):
                hi = 2 * pp + h
                nc.tensor.matmul(psos[h][:, :],
                                 v_sb[jt][:, hi:hi + 1, :],
                                 pt[:, h:h + 1, :],
                                 start=(jt == 0), stop=(jt == NJT - 1))
            if jt == NJT - 1:
                # free both PSUM accumulators right away; stage the rest
                osbs = {}
                for h in range(2):
                    o_sb = o_pool.tile([65, IC], F32, tag="osb",
                                       name=f"ob{pp}_{c}_{h}")
                    nc.vector.tensor_copy(o_sb[:, :], psos[h][:, :])
                    osbs[h] = o_sb
                for h in range(2):
                    ep_queue.append((pp, c, h, osbs[h], 0))
                    ep_queue.append((pp, c, h, osbs[h], 1))

        ep_queue = []
        ep_state = {}

        def run_ep_stage(pp, c, h, o_sb, stage):
            i0 = c * IC
            hi = 2 * pp + h
            if stage == 0:
                ps_t = aux_psum.tile([128, 512], F32, tag="aux", name="ps_t")
                for tq in range(IC // 128):
                    nc.tensor.transpose(ps_t[:, tq * 65:tq * 65 + 65],
                                        o_sb[:, tq * 128:(tq + 1) * 128],
                                        ident[0:65, 0:65])
                ep_state[(pp, c, h)] = ps_t
                return
            ps_t = ep_state.pop((pp, c, h))
            pst = ps_t[:, :]
            rc = r_pool.tile([128, 4], F32, tag="rc", name="rc")
            den = AP(pst.tensor, pst.offset + 64, [pst.ap[0], [65, 4]])
            nc.vector.reciprocal(rc[:, 0:4], den)
            ot = out_pool.tile([128, 4, HD], F32, tag="ot", name="ot")
            num = AP(pst.tensor, pst.offset, [pst.ap[0], [65, 4], [1, 64]])
            rcb = rc[:, :].unsqueeze(2).broadcast_to([128, 4, HD])
            nc.vector.tensor_mul(ot[:, :, :], num, rcb)
            ob = out_d[:, :]
            dst = AP(ob.tensor, ob.offset + i0 * 256 + hi * HD,
                     [[256, 128], [128 * 256, 4], [1, HD]])
            dma(hi + c, dst, ot[:, :, :])

        # late-projection schedule: unit index -> list of (kind, args).
        # Deadlines: kq(0,t) by unit 4t; kq(2,c) by unit 16c; vproj tt by
        # unit tt+1; kq(1,*)/kq(3,c) by unit 64+16c.
        sched = {1: [("kq", 0, 1)], 2: [("kq", 0, 2)], 3: [("kq", 0, 3)],
                 12: [("kq", 2, 1)], 26: [("kq", 2, 2)], 42: [("kq", 2, 3)],
                 20: [("kq", 1, 0)], 24: [("kq", 1, 1)],
                 34: [("kq", 1, 2)], 38: [("kq", 1, 3)],
                 50: [("kq", 3, 0)], 66: [("kq", 3, 1)],
                 72: [("kq", 3, 2)], 88: [("kq", 3, 3)]}

        units = [(pp, c, jt) for pp in range(2) for c in range(NCH)
                 for jt in range(NJT)]
        prev = None
        for idx, (pp, c, jt) in enumerate(units):
            ps = emit_S(pp, c, jt)
            if idx < NJT - 4:
                emit_vproj_tt(idx + 4)
            for item in sched.get(idx, ()):
                emit_kq_tcn(item[1], item[2])
            pt = emit_expmult(pp, c, jt, ps, nc.vector)
            if prev is not None:
                emit_AV(*prev)
            if ep_queue:
                run_ep_stage(*ep_queue.pop(0))
            prev = (pp, c, jt, pt)
        emit_AV(*prev)
        while ep_queue:
            run_ep_stage(*ep_queue.pop(0))

    nc.compile()
    return nc


def shard_inputs(inputs: dict) -> list[dict]:
    """Full inputs -> 8 per-core input maps (fp8/f32 prep for the device)."""
    import ml_dtypes

    x, w_in = inputs["x"], inputs["w_in"]
    off = inputs["kernel_offsets"]
    amp = inputs["kernel_amplitudes"]
    sh = inputs["kernel_sharpness"]
    D = DM
    in_maps = []
    for c in range(8):
        b, hg = c // 4, c % 4
        heads = list(range(4 * hg, 4 * hg + 4))
        # xT8[p, dc, t] = x[b, t, dc*128+p]
        xT8 = np.ascontiguousarray(
            x[b].T.reshape(NDC, 128, L).transpose(1, 0, 2)
        ).astype(ml_dtypes.bfloat16)
        rows_k = np.concatenate([w_in[h * HD:(h + 1) * HD] for h in heads])
        rows_q = np.concatenate(
            [w_in[2 * D + h * HD:2 * D + (h + 1) * HD] for h in heads])
        rows_v = np.concatenate([w_in[D + h * HD:D + (h + 1) * HD] for h in heads])
        wfeat = np.concatenate([rows_k, rows_q, rows_v])
        # w8[p, dc, e] = wfeat[e, dc*128+p]
        w8 = np.ascontiguousarray(
            wfeat.T.reshape(NDC, 128, 768).transpose(1, 0, 2)
        ).astype(ml_dtypes.bfloat16)
        tisa = np.zeros((64, 6), np.float32)
        tisa[:, 0] = -off[heads].reshape(-1)
        tisa[:, 1] = -np.abs(sh[heads].reshape(-1))
        for hi in range(4):
            tisa[hi * 16:(hi + 1) * 16, 2 + hi] = amp[heads[hi]]
        ramp = np.broadcast_to(
            (np.float32(L - 1) - np.arange(M, dtype=np.float32))[None, :],
            (64, M)).astype(ml_dtypes.bfloat16)
        in_maps.append({"xT": xT8, "wkqv": w8, "tisa": tisa, "ramp": ramp})
    return in_maps


def unshard_output(results: list[dict]) -> np.ndarray:
    out = np.zeros((2, L, DM), np.float32)
    for c in range(8):
        b, hg = c // 4, c % 4
        out[b, :, hg * 256:(hg + 1) * 256] = results[c]["out"]
    return out


_NC_CACHE = None


def kernel(**inputs) -> np.ndarray:
    global _NC_CACHE
    from concourse.bass_utils import run_bass_kernel_spmd

    if _NC_CACHE is None:
        _NC_CACHE = build_kernel()
    in_maps = shard_inputs({k: np.asarray(v) for k, v in inputs.items()})
    res = run_bass_kernel_spmd(_NC_CACHE, in_maps, core_ids=list(range(8)))
    return unshard_output(res.results)


# revision 15
# speedup vs baseline: 1.0871x; 1.0871x over previous
"""Trainium2 Bass kernel for nn_Attention_36361193128703 (self-contained).

Entry point: kernel(**inputs) -> np.ndarray
  inputs: x (2,2048,1024) f32, w_in (3072,1024) f32,
          kernel_offsets/amplitudes/sharpness (16,16) f32
  returns: (2, 2048, 1024) f32 attention output (matches reference).

Distribution: 8 NeuronCores = data-parallel over batch (2) x tensor-parallel
over heads (4 head-groups of 4). Each core runs an identical single-core Bass
program on its shard; outputs are concatenated on the host. No collectives.

Core pipeline (per core: 4 heads = 2 head-pairs, L=2048):
  - TISA scores (reversed) -> g = exp(bias) staged in DRAM; 128 shifted rows
    per head loaded with positive-stride DMA (Toeplitz expansion); the reversal
    moves to a stride -1 free-dim read in the bias multiply.
  - Projections in bf16 (fp8 tested: weight-quantization noise amplifies to
    ~4% output error, over the 2e-2 tolerance).
  - S = k^T q for a head-pair: two K=64 matmuls packed concurrently into PE
    row-groups 0-63 / 64-127, one [128, 2, 512] PSUM pair tile.
  - P = exp(S/8) * g: one ACT exp (FD=1024) + one DVE multiply (2x mode).
  - O = V^T P accumulated in PSUM with a ones-column denominator row.
  - Epilogue: PE transpose to [query, 64], DVE reciprocal+scale, DMA out.
"""
from contextlib import ExitStack

import numpy as np

import concourse.bass as bass
import concourse.mybir as mybir
import concourse.tile as tile
from concourse import bacc
from concourse.bass import AP
from concourse.masks import make_identity

F32 = mybir.dt.float32
BF16 = mybir.dt.bfloat16
FP8 = mybir.dt.float8e4
DR = mybir.MatmulPerfMode.DoubleRow

L = 2048
DM = 1024
HL = 4            # local heads
HD = 64
M = 4608          # padded score length (>= 2L-1 = 4095), 9 x 512 chunks
GW = 4096         # g window width per head (max needed index 4094)
IC = 512          # i-chunk (query) width per unit
NCH = L // IC     # 4 chunks
JT = 128          # j-tile (key) height
NJT = L // JT     # 16
NDC = DM // 128   # 8 d-chunks
NDP = NDC // 2    # 4 fp8 DoubleRow d-chunk pairs

# 1/sqrt(HD) is folded into the exp scale rather than the q weights.
QK_SCALE = 0.125
ONES_VAL = 1.0


def build_kernel() -> bacc.Bacc:
    nc = bacc.Bacc("TRN2", target_bir_lowering=False, debug=False, num_devices=8)

    xT_d = nc.dram_tensor("xT", [128, NDC, L], BF16, kind="ExternalInput")
    w_d = nc.dram_tensor("wkqv", [128, NDC, 768], BF16, kind="ExternalInput")
    tisa_d = nc.dram_tensor("tisa", [64, 6], F32, kind="ExternalInput")
    ramp_d = nc.dram_tensor("ramp", [64, 1024], F32, kind="ExternalInput")
    out_d = nc.dram_tensor("out", [L, 256], F32, kind="ExternalOutput")

    dma_engines = [nc.sync, nc.gpsimd]

    def dma(i, out, in_):
        dma_engines[i % len(dma_engines)].dma_start(out, in_)

    with tile.TileContext(nc) as tc, ExitStack() as ctx:
        const_pool = ctx.enter_context(tc.tile_pool(name="const", bufs=1))

        aux_psum = ctx.enter_context(tc.tile_pool(name="auxps", bufs=2, space="PSUM"))
        s_psum = ctx.enter_context(tc.tile_pool(name="sps", bufs=2, space="PSUM"))
        o_psum = ctx.enter_context(tc.tile_pool(name="ops", bufs=1, space="PSUM"))

        gdram_pool = ctx.enter_context(tc.tile_pool(name="gdram", bufs=1, space="DRAM"))
        g_band = gdram_pool.tile([HL * 1024], BF16)
        gbase = g_band[:]

        # ---------------- input DMAs -------------------------------------------
        # tisa first: it is tiny and gates the whole phase-0 chain.
        tp = ctx.enter_context(tc.tile_pool(name="tisa_tmp", bufs=1))
        tisa_sb = tp.tile([64, 6], F32)
        nc.sync.dma_start(tisa_sb[:, :], tisa_d[:, :])
        rsb = tp.tile([64, 1024], F32, tag="ramp")
        nc.gpsimd.dma_start(rsb[:, :], ramp_d[:, :])
        xpool = ctx.enter_context(tc.tile_pool(name="xT", bufs=1))
        wpool = ctx.enter_context(tc.tile_pool(name="w", bufs=1))
        xT_sb = xpool.tile([128, NDC, L], BF16, name="xT", tag="xT")
        w_sb = wpool.tile([128, NDC, 768], BF16, name="w", tag="w")
        for wc in range(4):
            dma(wc, w_sb[:, 2 * wc:2 * wc + 2, :], w_d[:, 2 * wc:2 * wc + 2, :])
        for dc in range(NDC):
            dma(dc, xT_sb[:, dc:dc + 1, 0:L // 2],
                xT_d[:, dc:dc + 1, 0:L // 2])
        for dc in range(NDC):
            dma(dc + 1, xT_sb[:, dc:dc + 1, L // 2:L],
                xT_d[:, dc:dc + 1, L // 2:L])

        # ---------------- Phase 0: TISA scores (reversed) -> g_pad ---------------
        # ev[:, m] = (L-1) - m so g_pad holds rev[m] = score[2L-2 - m]; the
        # Toeplitz flip then needs only positive DMA strides.
        # Band-windowed TISA: the Gaussian mixture has reach << L for these
        # params (host asserts max|off| + width < 255), so scores are computed
        # only on rev-band [1536, 2560); everywhere else exp(bias) == 1.0 in
        # bf16 and the g tiles are simply memset to 1.
        # tisa col0 = -off, col1 = -|sharp| (negated on the host), so the ACT
        # affine stage does the whole pointwise chain in two table-shared ops:
        # u = (ramp + (-off))^2, then evb = exp((-|sh|) * u).
        ev = tp.tile([64, 1024], F32, tag="scr")
        evb = tp.tile([64, 1024], BF16, tag="scrb")
        ampb = tp.tile([64, 4], BF16)
        nc.vector.tensor_copy(ampb[:, :], tisa_sb[:, 2:6])
        nc.scalar.activation(ev[:, :], rsb[:, :],
                             mybir.ActivationFunctionType.Square,
                             bias=tisa_sb[:, 0:1])
        nc.scalar.activation(evb[:, :], ev[:, :],
                             mybir.ActivationFunctionType.Exp,
                             scale=tisa_sb[:, 1:2])

        def emit_phase0_mms():
            gch = tp.tile([HL, 1024], BF16, tag="gch")
            for mc in range(2):
                ps = aux_psum.tile([128, 512], F32, tag="aux", name="ph0")
                nc.tensor.matmul(ps[0:HL, :], ampb[:, :],
                                 evb[:, mc * 512:(mc + 1) * 512],
                                 start=True, stop=True)
                nc.scalar.activation(gch[:, mc * 512:(mc + 1) * 512],
                                     ps[0:HL, :],
                                     mybir.ActivationFunctionType.Exp)
            dst = AP(gbase.tensor, gbase.offset, [[1024, HL], [1, 1024]])
            nc.sync.dma_start(dst, gch[:, :])

        # ---------------- projections ------------------------------------------
        kq_pool = ctx.enter_context(tc.tile_pool(name="kq", bufs=1))
        v_pool = ctx.enter_context(tc.tile_pool(name="V", bufs=1))
        kq_sb = [[kq_pool.tile([128, 512], BF16, name=f"kq{i}_{t}",
                          tag=f"kq{i}_{t}") for t in range(4)]
                 for i in range(4)]
        v_sb = [None] * NJT

        def emit_kq_tcn(ec, tcn):
            ps = aux_psum.tile([128, 512], F32, tag="aux", name="ps")
            for k in range(NDC):
                dc = (tcn * 2 + k) % NDC
                nc.tensor.matmul(ps[:, :],
                                 w_sb[:, dc:dc + 1, ec * 128:(ec + 1) * 128],
                                 xT_sb[:, dc:dc + 1, tcn * 512:(tcn + 1) * 512],
                                 start=(k == 0), stop=(k == NDC - 1))
            nc.vector.tensor_copy(kq_sb[ec][tcn][:, :], ps[:, :])

        def emit_vproj_tt(tt):
            ps = aux_psum.tile([128, 512], F32, tag="aux", name="ps")
            for dc in range(NDC):
                nc.tensor.matmul(ps[:, 0:256],
                                 xT_sb[:, dc:dc + 1, tt * 128:(tt + 1) * 128],
                                 w_sb[:, dc:dc + 1, 512:768],
                                 start=(dc == 0), stop=(dc == NDC - 1))
            vt = v_pool.tile([128, HL, 65], BF16, name=f"v{tt}", tag=f"v{tt}")
            psa = ps[:, 0:256]
            ps3 = AP(psa.tensor, psa.offset, [psa.ap[0], [64, HL], [1, 64]])
            nc.vector.tensor_copy(vt[:, :, 0:64], ps3)
            nc.gpsimd.memset(vt[:, :, 64:65], ONES_VAL)
            v_sb[tt] = vt

        # k/q for head-pair 0 tokens 0-511 gate the first unit; everything else
        # interleaves into the unit stream (see schedule below).
        emit_kq_tcn(0, 0)
        emit_kq_tcn(2, 0)
        emit_phase0_mms()
        for tt in range(4):
            emit_vproj_tt(tt)
        ident = const_pool.tile([128, 128], F32)
        make_identity(nc, ident[:, :])

        # ---------------- Phase 2: g windows (positive-stride DMA) ---------------
        grep_pool = ctx.enter_context(tc.tile_pool(name="grep", bufs=1))
        grepp = []
        for pp in range(2):
            gt = grep_pool.tile([128, 2, GW], BF16, name=f"grep{pp}",
                                tag=f"grep{pp}")
            nc.vector.memset(gt[:, :, :], 1.0)
            grepp.append(gt)
        for hi in range(HL):
            src = AP(gbase.tensor, gbase.offset + hi * 1024 + 129,
                     [[1, 128], [1, 640]])
            dma(hi, grepp[hi // 2][:, hi % 2:hi % 2 + 1, 1665:2305], src)

        # ---------------- Phase 3: attention units -------------------------------
        p_pool = ctx.enter_context(tc.tile_pool(name="p", bufs=6))
        e_pool = ctx.enter_context(tc.tile_pool(name="es", bufs=6))
        o_pool = ctx.enter_context(tc.tile_pool(name="o", bufs=2))
        r_pool = ctx.enter_context(tc.tile_pool(name="r", bufs=2))
        out_pool = ctx.enter_context(tc.tile_pool(name="out", bufs=2))

        def emit_S(pp, c, jt):
            """Row-packed head-pair S matmuls -> [128, 2, 512] PSUM tile."""
            ps = s_psum.tile([128, 2, IC], F32, tag="S", name=f"s{pp}_{c}_{jt}")
            kqt = kq_sb[pp][jt // 4]
            qqt = kq_sb[2 + pp][c]
            joff = (jt % 4) * JT
            for h in range(2):
                pb = h * 64
                nc.tensor.matmul(ps[:, h:h + 1, :],
                                 kqt[pb:pb + 64, joff:joff + JT],
                                 qqt[pb:pb + 64, :],
                                 start=True, stop=True)
            return ps

        def emit_expmult(pp, c, jt, ps, eng):
            es = e_pool.tile([128, 2, IC], BF16, tag="es", name=f"e{pp}_{c}_{jt}")
            nc.scalar.activation(es[:, :, :], ps[:, :, :],
                                 mybir.ActivationFunctionType.Exp,
                                 scale=QK_SCALE)
            pt = p_pool.tile([128, 2, IC], BF16, tag="p", name=f"p{pp}_{c}_{jt}")
            u0 = (L - 1) - c * IC + jt * JT
            g2 = grepp[pp][:, :, :]
            g_rev = AP(g2.tensor, g2.offset + u0, [g2.ap[0], [GW, 2], [-1, IC]])
            eng.tensor_mul(pt[:, :, :], es[:, :, :], g_rev)
            return pt

        psos = {}

        def emit_AV(pp, c, jt, pt):
            if jt == 0:
                psos[0] = o_psum.tile([65, IC], F32, tag="O0", name=f"o0_{pp}_{c}")
                psos[1] = o_psum.tile([65, IC], F32, tag="O1", name=f"o1_{pp}_{c}")
            for h in range(2):
                hi = 2 * pp + h
                nc.tensor.matmul(psos[h][:, :],
                                 v_sb[jt][:, hi:hi + 1, :],
                                 pt[:, h:h + 1, :],
                                 start=(jt == 0), stop=(jt == NJT - 1))
            if jt == NJT - 1:
                emit_epilogue(pp, c)

        def emit_epilogue(pp, c):
            i0 = c * IC
            for h in range(2):
                hi = 2 * pp + h
                o_sb = o_pool.tile([65, IC], F32, tag="osb", name=f"ob{pp}_{c}_{h}")
                nc.vector.tensor_copy(o_sb[:, :], psos[h][:, :])
                # 4 transposed [128, 65] chunks packed into one aux psum bank
                ps_t = aux_psum.tile([128, 512], F32, tag="aux", name="ps_t")
                for tq in range(IC // 128):
                    nc.tensor.transpose(ps_t[:, tq * 65:tq * 65 + 65],
                                        o_sb[:, tq * 128:(tq + 1) * 128],
                                        ident[0:65, 0:65])
                rc = r_pool.tile([128, 4], F32, tag="rc", name="rc")
                pst = ps_t[:, :]
                den = AP(pst.tensor, pst.offset + 64, [pst.ap[0], [65, 4]])
                nc.vector.reciprocal(rc[:, 0:4], den)
                ot = out_pool.tile([128, 4, HD], F32, tag="ot", name="ot")
                num = AP(pst.tensor, pst.offset, [pst.ap[0], [65, 4], [1, 64]])
                rcb = rc[:, :].unsqueeze(2).broadcast_to([128, 4, HD])
                nc.vector.tensor_mul(ot[:, :, :], num, rcb)
                ob = out_d[:, :]
                dst = AP(ob.tensor, ob.offset + i0 * 256 + hi * HD,
                         [[256, 128], [128 * 256, 4], [1, HD]])
                dma(hi + c, dst, ot[:, :, :])

        # late-projection schedule: unit index -> list of (kind, args).
        # Deadlines: kq(0,t) by unit 4t; kq(2,c) by unit 16c; vproj tt by
        # unit tt+1; kq(1,*)/kq(3,c) by unit 64+16c.
        sched = {1: [("kq", 0, 1)], 2: [("kq", 0, 2)], 3: [("kq", 0, 3)],
                 12: [("kq", 2, 1)], 26: [("kq", 2, 2)], 42: [("kq", 2, 3)],
                 20: [("kq", 1, 0)], 24: [("kq", 1, 1)],
                 34: [("kq", 1, 2)], 38: [("kq", 1, 3)],
                 50: [("kq", 3, 0)], 66: [("kq", 3, 1)],
                 72: [("kq", 3, 2)], 88: [("kq", 3, 3)]}

        units = [(pp, c, jt) for pp in range(2) for c in range(NCH)
                 for jt in range(NJT)]
        prev = None
        for idx, (pp, c, jt) in enumerate(units):
            ps = emit_S(pp, c, jt)
            if idx < NJT - 4:
                emit_vproj_tt(idx + 4)
            for item in sched.get(idx, ()):
                emit_kq_tcn(item[1], item[2])
            pt = emit_expmult(pp, c, jt, ps, nc.vector)
            if prev is not None:
                emit_AV(*prev)
            prev = (pp, c, jt, pt)
        emit_AV(*prev)

    nc.compile()
    return nc


def shard_inputs(inputs: dict) -> list[dict]:
    """Full inputs -> 8 per-core input maps (fp8/f32 prep for the device)."""
    import ml_dtypes

    x, w_in = inputs["x"], inputs["w_in"]
    off = inputs["kernel_offsets"]
    amp = inputs["kernel_amplitudes"]
    sh = inputs["kernel_sharpness"]
    D = DM
    in_maps = []
    for c in range(8):
        b, hg = c // 4, c % 4
        heads = list(range(4 * hg, 4 * hg + 4))
        # xT8[p, dc, t] = x[b, t, dc*128+p]
        xT8 = np.ascontiguousarray(
            x[b].T.reshape(NDC, 128, L).transpose(1, 0, 2)
        ).astype(ml_dtypes.bfloat16)
        rows_k = np.concatenate([w_in[h * HD:(h + 1) * HD] for h in heads])
        rows_q = np.concatenate(
            [w_in[2 * D + h * HD:2 * D + (h + 1) * HD] for h in heads])
        rows_v = np.concatenate([w_in[D + h * HD:D + (h + 1) * HD] for h in heads])
        wfeat = np.concatenate([rows_k, rows_q, rows_v])
        # w8[p, dc, e] = wfeat[e, dc*128+p]
        w8 = np.ascontiguousarray(
            wfeat.T.reshape(NDC, 128, 768).transpose(1, 0, 2)
        ).astype(ml_dtypes.bfloat16)
        tisa = np.zeros((64, 6), np.float32)
        tisa[:, 0] = -off[heads].reshape(-1)
        tisa[:, 1] = -np.abs(sh[heads].reshape(-1))
        for hi in range(4):
            tisa[hi * 16:(hi + 1) * 16, 2 + hi] = amp[heads[hi]]
        reach = np.abs(off).max() + np.sqrt(
            np.log(max(np.abs(amp).max(), 1e-3) / 1e-3)
            / max(np.abs(sh).min(), 1e-4))
        assert reach < 255.0, f"TISA reach {reach} exceeds band window"
        # rev-band m in [1536, 2560): rel = 2047 - m = 511 - j
        ramp = np.broadcast_to(
            (np.float32(511) - np.arange(1024, dtype=np.float32))[None, :],
            (64, 1024)).copy()
        in_maps.append({"xT": xT8, "wkqv": w8, "tisa": tisa, "ramp": ramp})
    return in_maps


def unshard_output(results: list[dict]) -> np.ndarray:
    out = np.zeros((2, L, DM), np.float32)
    for c in range(8):
        b, hg = c // 4, c % 4
        out[b, :, hg * 256:(hg + 1) * 256] = results[c]["out"]
    return out


_NC_CACHE = None


def kernel(**inputs) -> np.ndarray:
    global _NC_CACHE
    from concourse.bass_utils import run_bass_kernel_spmd

    if _NC_CACHE is None:
        _NC_CACHE = build_kernel()
    in_maps = shard_inputs({k: np.asarray(v) for k, v in inputs.items()})
    res = run_bass_kernel_spmd(_NC_CACHE, in_maps, core_ids=list(range(8)))
    return unshard_output(res.results)
